# revision 20
# baseline (speedup 1.0000x reference)
"""CubicFeatureSampling Trainium2 kernel (v3).

Problem (hardcoded shapes):
  ptcloud        [B=4, N=16384, 3]  f32 in [-1, 1]
  cubic_features [B=4, C=128, S=32, S, S] f32
  neighborhood_size = 1  (V = 8 cell-corner vertices)
  output         [B, N, V=8, C=128] f32
      out[b,n,v,c] = cf[b,c, fx+dx, fy+dy, fz+dz]  (v = dx*4+dy*2+dz)
      where (fx,fy,fz) = floor(pt*16+16), zero when any corner coord >= 32.

Sharding: 8 cores = (batch b = core//2, half of N = core%2); 8192 points/core.

All on-device data is bf16 (rel tolerance 2e-2 >> bf16 round-off ~2e-3):
host converts cf to bf16 (zero-padded by 64 cols), device writes bf16 out,
host upcasts to f32.

Phase 1 builds a 4x-duplicated corner table in DRAM:
  T8 row r = y*1024 + z*32 + x  (1KB rows, 512 bf16)
  T8[r] = [cf(x,y,z), cf(x,y,z+1), cf(x,y+1,z), cf(x,y+1,z+1)]   ((dy,dz,c))
so ONE gather element (2KB = rows r, r+1, overlap window along x) yields all
8 corners of cell (x,y,z) in output-row order (dx,dy,dz,c). Built with four
PE transposes per 128-block at shifts {0,1,32,33}: the shifted slice IS the
(y+dy, z+dz) neighbor. With y-major rows the store's partition map (yl,z) is
a single uniform-stride dim and runs are 2KB-contiguous (line rate). PSUM ->
SBUF copies alternate DVE/ACT.

Phase 2: 4 dma_gather calls x 2048 idxs (1 descriptor per point -- Q7
descriptor generation ~16ns/desc was the baseline bottleneck). The index
list is permuted so gather-out partition p slot s holds point p*16+s, making
each store ONE fully contiguous-per-partition 4MB dma_start. Point loads are
replicated across the 8 Q7 groups with a single PE broadcast matmul instead
of 8 DMAs. Out-of-range corners gather in-table garbage (finite reals) or the
explicitly zeroed tail row; the validity mask (computed on DVE in gather-out
layout) zeroes them.
"""

import numpy as np
import ml_dtypes

B, N, C, S = 4, 16384, 128, 32
V = 8
NCORES = 8
HALF = N // 2            # 8192 points per core
ROWS = S * S * S         # 32768 table rows (idx fits int16)
NCALL = 4
NPTS = HALF // NCALL     # 2048 points (= gather idxs) per call
SLOTS = NPTS // 128      # 16 slots per partition per call
XPLANE = S * S           # 1024 spatial positions per x-plane
CHUNK = 2 * XPLANE       # phase-1 chunk: 2 x-planes = 2048 columns
PAD = 64                 # host-side zero pad columns on cf (shifted reads)
NCHUNK = ROWS // CHUNK   # 16


def _build(loops: int, variant: str = "full"):
    import concourse.bacc as bacc
    import concourse.bass as bass
    import concourse.mybir as mybir
    import concourse.tile as tile
    from concourse.masks import make_identity

    f32 = mybir.dt.float32
    bf16 = mybir.dt.bfloat16
    i16 = mybir.dt.int16
    Alu = mybir.AluOpType
    Act = mybir.ActivationFunctionType

    prep = variant == "prep"
    nc = bacc.Bacc(
        "TRN2",
        target_bir_lowering=False,
        dynamic_dma_scratch_size=24576 if prep else 16384,
    )
    cfb = nc.declare_dram_parameter("cfb", [C, ROWS + PAD], bf16, isOutput=False)
    pt = nc.declare_dram_parameter("pt", [HALF, 3], f32, isOutput=False)
    dc = nc.declare_dram_parameter("dc", [128, 2], f32, isOutput=False)
    rp = nc.declare_dram_parameter("rp", [16, 128], f32, isOutput=False)
    out = nc.declare_dram_parameter("out", [HALF * V, C], bf16, isOutput=True)
    t8 = nc.dram_tensor("t8", [ROWS + 2, 4 * C], bf16)

    with tile.TileContext(nc) as tc:
        with (
            tc.tile_pool(name="const", bufs=1) as constp,
            tc.tile_pool(name="grid", bufs=2) as gridp,
            tc.tile_pool(name="stage", bufs=1 if prep else 2) as stagep,
            tc.tile_pool(name="psum", bufs=3, space="PSUM") as psump,
            tc.tile_pool(name="pspt", bufs=2, space="PSUM") as psptp,
            tc.tile_pool(name="idxp", bufs=2) as idxp,
            tc.tile_pool(name="keep", bufs=1) as keepp,
            tc.tile_pool(name="gat", bufs=4 if prep else 3) as gatp,
        ):
            ident = constp.tile([128, 128], bf16)
            make_identity(nc, ident[:])
            dct = constp.tile([128, 2], f32)
            nc.sync.dma_start(out=dct[:], in_=dc[:])
            d01 = dct[:, 0:2]                      # [0, 1] along free dim
            rpt = constp.tile([16, 128], f32)
            nc.sync.dma_start(out=rpt[:], in_=rp[:])
            zrow = constp.tile([128, 4], bf16)
            nc.vector.memset(zrow[:], 0.0)

            def body():
                # zero the tail row (read by idx 32767's overlap window)
                nc.scalar.dma_start(
                    out=t8[ROWS : ROWS + 1, :].rearrange(
                        "o (p w) -> (o p) w", p=128
                    ),
                    in_=zrow[:],
                )
                # ---------- phase 1: build 4x-dup corner table ----------
                for q in range(0 if variant in ("nophase1", "gonly") else NCHUNK):
                    chunk = gridp.tile([128, CHUNK + PAD], bf16, tag="chunk")
                    nc.sync.dma_start(
                        out=chunk[:], in_=cfb[:, q * CHUNK : (q + 1) * CHUNK + PAD]
                    )
                    stag = stagep.tile([128, (CHUNK // 128) * 512], bf16, tag="stag")
                    sv2 = stag[:].rearrange(
                        "p (yh ii w) -> p yh ii w", ii=2, w=512
                    )
                    for g in range(4):
                        ps = psump.tile([128, 2048], bf16, space="PSUM")
                        for j in range(4):
                            t = 4 * g + j
                            for pos, sh in enumerate((0, 1, 32, 33)):
                                nc.tensor.transpose(
                                    out=ps[:, (j * 4 + pos) * 128 : (j * 4 + pos + 1) * 128],
                                    in_=chunk[:, t * 128 + sh : t * 128 + sh + 128],
                                    identity=ident[:],
                                )
                        # t = ii*8 + yh: group g covers ii = g//2,
                        # yh in [4*(g%2), 4*(g%2)+4)
                        dstv = sv2[:, 4 * (g % 2) : 4 * (g % 2) + 4, g // 2, :]
                        if (q * 4 + g) % 2 == 0:
                            nc.vector.tensor_copy(out=dstv, in_=ps[:])
                        else:
                            nc.scalar.copy(out=dstv, in_=ps[:])
                    # table rows r = (yh*4+yl)*1024 + z*32 + (q*2+ii):
                    # partition p=(yl,z) has uniform DRAM stride 32 rows.
                    dst = t8[0:ROWS, :].rearrange(
                        "(yh yl z xq xi) w -> xq yl z yh (xi w)",
                        yh=8, yl=4, z=32, xq=16, xi=2,
                    )[q]
                    eng = nc.sync if q % 2 == 0 else nc.scalar
                    eng.dma_start(
                        out=dst,
                        in_=stag[:].rearrange("p (yh iw) -> p yh iw", iw=1024),
                    )

                # ---------- phase 2: gather ----------
                t8_ap = t8[:]
                gather_src = bass.AP(t8_ap.tensor, 0, [[4 * C, ROWS], [1, 8 * C]])

                percall = []
                for k in range(NCALL):
                    nlo = k * NPTS
                    # ptq: partition q holds floors of points n = f*256+q*16+s
                    # (free = (f8, s16, c3)); PE-broadcast to all 8 groups.
                    ptq = idxp.tile([16, 8 * 16 * 3], f32, tag="ptq")
                    nc.sync.dma_start(
                        out=ptq[:].rearrange("q (f s c) -> q f s c", f=8, c=3),
                        in_=pt[nlo : nlo + NPTS, :].rearrange(
                            "(f q s) c -> q f s c", f=8, q=16
                        ),
                    )
                    pspt = psptp.tile([128, 8 * 16 * 3], f32, space="PSUM")
                    nc.tensor.matmul(
                        out=pspt[:], lhsT=rpt[:], rhs=ptq[:],
                    )
                    # ptm: partition p holds floors of points n = p*16 + s
                    ptm = idxp.tile([128, 16 * 3], f32, tag="ptm")
                    nc.sync.dma_start(
                        out=ptm[:].rearrange("p (s c) -> p s c", c=3),
                        in_=pt[nlo : nlo + NPTS, :].rearrange(
                            "(p s) c -> p s c", p=128
                        ),
                    )

                    # exact floor: fl = round(t) - (round(t) > t);
                    # t = pt*16+16 computed on ACT straight out of PSUM/SBUF
                    def floor_tiles(src_ap, width, tag):
                        t_ = idxp.tile([128, width], f32, tag=f"t{tag}")
                        nc.scalar.activation(
                            out=t_[:], in_=src_ap, func=Act.Copy,
                            scale=16.0, bias=16.0,
                        )
                        r_ = idxp.tile([128, width], f32, tag=f"r{tag}")
                        nc.vector.tensor_scalar(
                            out=r_[:], in0=t_[:], scalar1=float(2 ** 23),
                            scalar2=-float(2 ** 23), op0=Alu.add, op1=Alu.add,
                        )
                        g_ = idxp.tile([128, width], f32, tag=f"g{tag}")
                        nc.vector.tensor_tensor(
                            out=g_[:], in0=r_[:], in1=t_[:], op=Alu.is_gt
                        )
                        f_ = idxp.tile([128, width], f32, tag=f"f{tag}")
                        nc.vector.tensor_tensor(
                            out=f_[:], in0=r_[:], in1=g_[:], op=Alu.subtract
                        )
                        return f_

                    fl = floor_tiles(pspt[:], 8 * 16 * 3, "w")
                    flm = floor_tiles(ptm[:], 16 * 3, "m")
                    flmv = flm[:].rearrange("p (s c) -> p s c", c=3)

                    # clamped coords (floors are >= 0 here; clamp top only)
                    cl = idxp.tile([128, 8 * 16 * 3], f32, tag="cl")
                    nc.vector.tensor_scalar(
                        out=cl[:], in0=fl[:], scalar1=31.0, scalar2=None,
                        op0=Alu.min,
                    )
                    clv = cl[:].rearrange("p (f s c) -> p f s c", f=8, c=3)

                    # idx[q, s*8+f] = y*1024 + z*32 + x  of point (f,q,s)
                    # free iteration (s, f): transposed view of (f, s) data
                    rowf = idxp.tile([128, SLOTS * 8], f32, tag="rowf")
                    rv = rowf[:].rearrange("p (s f) -> p s f", f=8)
                    zt_v = clv[:, :, :, 2].rearrange("p f s -> p s f")
                    yt_v = clv[:, :, :, 1].rearrange("p f s -> p s f")
                    xt_v = clv[:, :, :, 0].rearrange("p f s -> p s f")
                    nc.vector.scalar_tensor_tensor(
                        out=rv, in0=yt_v, scalar=float(S), in1=zt_v,
                        op0=Alu.mult, op1=Alu.add,
                    )
                    nc.vector.scalar_tensor_tensor(
                        out=rv, in0=rv, scalar=float(S), in1=xt_v,
                        op0=Alu.mult, op1=Alu.add,
                    )
                    wk = keepp.tile([128, SLOTS * 8], i16, tag=f"wk{k}")
                    nc.vector.tensor_copy(out=wk[:], in_=rowf[:])

                    # validity mask m8[p, s, dx, dy, dz] for point p*16+s
                    def axmask(coord_ap, tag):
                        m_ = idxp.tile([128, SLOTS * 2], f32, tag=f"m{tag}")
                        mv = m_[:].rearrange("p (s d) -> p s d", d=2)
                        nc.vector.tensor_tensor(
                            out=mv,
                            in0=coord_ap[:, :, None].broadcast_to([128, SLOTS, 2]),
                            in1=d01[:, None, :].broadcast_to([128, SLOTS, 2]),
                            op=Alu.add,
                        )
                        nc.vector.tensor_scalar(
                            out=m_[:], in0=m_[:], scalar1=float(S), scalar2=None,
                            op0=Alu.is_lt,
                        )
                        return m_

                    mx = axmask(flmv[:, :, 0], "x")
                    my = axmask(flmv[:, :, 1], "y")
                    mz = axmask(flmv[:, :, 2], "z")
                    mxy = idxp.tile([128, SLOTS * 4], f32, tag="mxy")
                    nc.vector.tensor_tensor(
                        out=mxy[:].rearrange("p (s a b) -> p s a b", a=2, b=2),
                        in0=mx[:].rearrange("p (s a) -> p s a", a=2)[
                            :, :, :, None
                        ].broadcast_to([128, SLOTS, 2, 2]),
                        in1=my[:].rearrange("p (s b) -> p s b", b=2)[
                            :, :, None, :
                        ].broadcast_to([128, SLOTS, 2, 2]),
                        op=Alu.mult,
                    )
                    m8 = keepp.tile([128, SLOTS * 8], bf16, tag=f"m8{k}")
                    nc.vector.tensor_tensor(
                        out=m8[:].rearrange("p (s ab d) -> p s ab d", ab=4, d=2),
                        in0=mxy[:].rearrange("p (s ab) -> p s ab", ab=4)[
                            :, :, :, None
                        ].broadcast_to([128, SLOTS, 4, 2]),
                        in1=mz[:].rearrange("p (s d) -> p s d", d=2)[
                            :, :, None, :
                        ].broadcast_to([128, SLOTS, 4, 2]),
                        op=Alu.mult,
                    )

                    percall.append((wk, m8))

                # gather: elem = 2 rows (2KB) = all 8 corners of the cell.
                # Separate loop so call k+1's DVE work never sits behind
                # call k's drain-stalled mask-mult in the in-order queue.
                for k in range(NCALL):
                    wk, m8 = percall[k]
                    gt = gatp.tile([128, SLOTS * 8 * C], bf16, tag="g")
                    if variant != "nogather":
                        if prep:
                            gsem = nc.alloc_semaphore(f"gsem{k}")
                            nc.gpsimd.dma_gather(
                                out_ap=gt[:].rearrange(
                                    "p (s e) -> p s e", e=8 * C
                                ),
                                in_ap=gather_src,
                                idxs_ap=wk[:],
                                num_idxs=NPTS,
                                num_idxs_reg=NPTS,
                                elem_size=8 * C,
                                elem_step=4 * C,
                                single_packet=False,
                                prepare_only=True,
                                sem=gsem,
                            )
                        else:
                            nc.gpsimd.dma_gather(
                                out_ap=gt[:].rearrange(
                                    "p (s e) -> p s e", e=8 * C
                                ),
                                in_ap=gather_src,
                                idxs_ap=wk[:],
                                num_idxs=NPTS,
                                num_idxs_reg=NPTS,
                                elem_size=8 * C,
                                elem_step=4 * C,
                                single_packet=False,
                            )
                    percall[k] = (gt, m8)

                if prep and variant != "nogather":
                    nc.gpsimd.trigger_dma(count=None)
                if variant == "gonly":
                    return
                for k in range(NCALL):
                    gt, m8 = percall[k]
                    # zero out-of-range corners
                    nc.vector.tensor_tensor(
                        out=gt[:].rearrange("p (sv c) -> p sv c", c=C),
                        in0=gt[:].rearrange("p (sv c) -> p sv c", c=C),
                        in1=m8[:][:, :, None].broadcast_to(
                            [128, SLOTS * 8, C]
                        ),
                        op=Alu.mult,
                    )
                    # store: point p*16+s -> rows (k*2048 + p*16 + s)*8 .. +8
                    ov = out[k * NPTS * V : (k + 1) * NPTS * V, :].rearrange(
                        "(p s v) c -> p (s v c)", p=128, v=V
                    )
                    eng = nc.sync if k % 2 == 0 else nc.scalar
                    eng.dma_start(out=ov, in_=gt[:])

            if loops == 1:
                body()
            else:
                with tc.For_i(0, loops, 1):
                    body()

    nc.compile()
    return nc


def _in_maps(ptcloud: np.ndarray, cubic_features: np.ndarray):
    dconst = np.zeros((128, 2), np.float32)
    dconst[:, 1] = 1.0
    repmat = (np.arange(128)[None, :] % 16 == np.arange(16)[:, None]).astype(
        np.float32
    )
    cf_flat = cubic_features.reshape(B, C, ROWS).astype(ml_dtypes.bfloat16)
    cf_pad = np.zeros((B, C, ROWS + PAD), ml_dtypes.bfloat16)
    cf_pad[:, :, :ROWS] = cf_flat
    maps = []
    for core in range(NCORES):
        b, h = core // 2, core % 2
        maps.append(
            {
                "cfb": cf_pad[b],
                "pt": np.ascontiguousarray(ptcloud[b, h * HALF : (h + 1) * HALF]),
                "dc": dconst,
                "rp": repmat,
            }
        )
    return maps


_NC_CACHE: dict = {}


def get_nc(loops: int = 1, variant: str = "full"):
    key = (loops, variant)
    if key not in _NC_CACHE:
        _NC_CACHE[key] = _build(loops, variant)
    return _NC_CACHE[key]


def run_on_cores(in_maps, loops: int = 1, variant: str = "full", **kw):
    from concourse.bass_utils import run_bass_kernel_spmd

    nc = get_nc(loops, variant)
    return run_bass_kernel_spmd(nc, in_maps, list(range(NCORES)), **kw)


def kernel(ptcloud, cubic_features, neighborhood_size) -> np.ndarray:
    assert int(neighborhood_size) == 1
    ptcloud = np.asarray(ptcloud, dtype=np.float32)
    cubic_features = np.asarray(cubic_features, dtype=np.float32)
    assert ptcloud.shape == (B, N, 3)
    assert cubic_features.shape == (B, C, S, S, S)

    res = run_on_cores(_in_maps(ptcloud, cubic_features)).results
    outa = np.empty((B, N, V, C), np.float32)
    for core in range(NCORES):
        b, h = core // 2, core % 2
        outa[b, h * HALF : (h + 1) * HALF] = (
            res[core]["out"].astype(np.float32).reshape(HALF, V, C)
        )
    return outa
